# revision 1
# baseline (speedup 1.0000x reference)
"""GAT layer kernel for Trainium2 (8 NeuronCores, SPMD) — bf16 pipeline.

Math note: the per-destination softmax weights are only used through their
mean over each destination's incoming edges, and a softmax sums to 1, so
attn_w[i] = 1/deg[i] (0 if deg==0) exactly.  The output reduces to:

    out[i] = (agg[i] @ Wv.T + deg[i]*bv) * recip[i],  agg[i] = sum x[row[e]]

Device strategy (dst-node sharded, 49 windows of 128 dst nodes per core):
  - host sorts edges by (dst window, src half) and packs each window's
    edge list into T = T_LO + T_HI chunks of 128 slots; x is bf16.
  - per group of G windows: FOUR dma_gather calls (int16 indices, x
    split into two <32768-row halves) fetch x[row[e]] rows (256B each)
    into SBUF [128 slot, chunks*128] bf16.  The four gathers are spread
    over SWDGE queues 1,2,3,0: descriptor generation (~8ns/row) runs on
    a queue's own Q7 core pair, so queues 1-3 retire instantly on the
    Pool engine and generate concurrently; only queue 0 blocks.  Each
    DMASW sem lane is post-compile pinned to a single queue (ucode
    requirement).  Deep tile buffering (bufs=6) keeps the four
    generators saturated; this 4-way descriptor generation is the
    critical path (~24us per group).
  - per window one wide DVE op builds all T bf16 one-hots
    oh[p, t*128+j] = (j == col_local[p, t]); TensorE accumulates
    aggT[din, dst] += Xg_t^T @ oh_t into PSUM with bf16 matmuls.
  - epilogue (bf16 weights): out[dst, :] = (aggT^T @ WvT + deg^T x bv)
    * recip[dst], with the PSUM->SBUF copy and the recip scale on the
    Scalar engine.
"""

import os
import numpy as np

P = 128
NCORES = 8
N = 50000
XLO = 25088                   # rows in the low half of x (< 32768 for int16)
XHI = N - XLO
DIN = 128
DOUT = 128
WPC = 49                      # windows per core
NWIN = NCORES * WPC           # 392
NPAD = NWIN * P               # 50176
G = 5                         # windows per gather group

_last_exec_ns = None
_cache = {}


def _groups():
    # tapered tail: the final 1-window group drains the pipeline in ~6us
    # (gather gen + DMA + compute) instead of a full group's ~24us
    sizes = [G] * 9 + [3, 1]
    assert sum(sizes) == WPC
    out = []
    g0 = 0
    for s in sizes:
        out.append((g0, s))
        g0 += s
    return out


def _gsplits(Gg, T_LO, T_HI):
    """Per-group gather splits: (src_half, chunk_base, c0, c1, queue)."""
    nl = Gg * T_LO
    nh = Gg * T_HI
    la = (nl + 1) // 2
    ha = (nh + 1) // 2
    return [
        (0, 0, 0, la, 1),
        (0, 0, la, nl, 2),
        (1, nl, 0, ha, 3),
        (1, nl, ha, nh, 0),
    ]


def _ensure_ntff_hook():
    import sys
    import types
    if "antenv.axon_hooks" in sys.modules:
        return
    try:
        import antenv
        mod = types.ModuleType("antenv.axon_hooks")
        _h = [None]
        mod.set_axon_ntff_profile_hook = lambda hook: _h.__setitem__(0, hook)
        mod.get_axon_ntff_profile_hook = lambda: _h[0]
        sys.modules["antenv.axon_hooks"] = mod
        antenv.axon_hooks = mod
        from trn_agent_boot.trn_boot import _ntff_profile_via_ctypes
        hook = _ntff_profile_via_ctypes("/opt/axon/libaxon_pjrt.so")
        if hook is not None:
            mod.set_axon_ntff_profile_hook(hook)
    except Exception:
        pass


def _offsets(T):
    """Column offsets of the packed [P, CW] f32 constant tensor."""
    o = {}
    o["idx16"] = 0                        # int16 idx (wrapped), WPC*T*4 f32
    o["colb"] = o["idx16"] + WPC * T * 4  # col_local bf16, WPC*T/2 cols
    o["rec"] = o["colb"] + (WPC * T + 1) // 2
    o["wvtb"] = o["rec"] + WPC            # Wv.T bf16, DOUT/2 cols
    o["iotab"] = o["wvtb"] + DOUT // 2    # iota tiled bf16, T*P/2 cols
    o["bvb"] = o["iotab"] + T * P // 2    # bv bf16 at partition 0
    o["degb"] = o["bvb"] + DOUT // 2      # deg bf16 at partition 0
    o["CW"] = o["degb"] + WPC * P // 2
    return o


def _build(T, T_LO, T_HI):
    import concourse.bacc as bacc
    import concourse.mybir as mybir
    from concourse.tile import TileContext

    f32 = mybir.dt.float32
    bf16 = mybir.dt.bfloat16
    i16 = mybir.dt.int16

    o = _offsets(T)
    CW = o["CW"]

    nc = bacc.Bacc(None, target_bir_lowering=False, num_swdge_queues=4)
    xlo_d = nc.dram_tensor("xlo", [XLO, DIN], bf16, kind="ExternalInput")
    xhi_d = nc.dram_tensor("xhi", [XHI, DIN], bf16, kind="ExternalInput")
    NIDX = WPC * T * 4
    cidx_d = nc.dram_tensor("cidx", [P, NIDX], f32, kind="ExternalInput")
    crest_d = nc.dram_tensor("crest", [P, CW - NIDX], f32,
                             kind="ExternalInput")
    out_d = nc.dram_tensor("out", [WPC * P, DOUT], f32, kind="ExternalOutput")

    with TileContext(nc) as tc:
        with (
            tc.tile_pool(name="const", bufs=1) as cpool,
            tc.tile_pool(name="xg", bufs=6) as xgpool,
            tc.tile_pool(name="oh", bufs=6) as ohpool,
            tc.tile_pool(name="ep", bufs=2) as eppool,
            tc.tile_pool(name="ps", bufs=2, space="PSUM") as pspool,
            tc.tile_pool(name="po", bufs=2, space="PSUM") as popool,
        ):
            cidx_sb = cpool.tile([P, NIDX], f32, tag="cidx")
            crest_sb = cpool.tile([P, CW - NIDX], f32, tag="crest")
            # idx table first (the gathers only need this), rest loads in
            # the shadow of the first gather's descriptor generation; both
            # on HWDGE (sync) so the SWDGE lane round-robin stays aligned
            # with the periodic gather queue cycle below
            nc.sync.dma_start(out=cidx_sb[:], in_=cidx_d[:, :])
            nc.sync.dma_start(out=crest_sb[:], in_=crest_d[:, :])

            r = NIDX
            idx16_sb = cidx_sb[:].bitcast(i16)
            colb_sb = crest_sb[:, o["colb"] - r:o["rec"] - r].bitcast(bf16)
            rec_sb = crest_sb[:, o["rec"] - r:o["rec"] - r + WPC]
            wvtb_sb = crest_sb[:, o["wvtb"] - r:
                               o["wvtb"] - r + DOUT // 2].bitcast(bf16)
            iotab_sb = crest_sb[:, o["iotab"] - r:
                                o["iotab"] - r + T * P // 2].bitcast(bf16)
            bvb_sb = crest_sb[0:1, o["bvb"] - r:
                              o["bvb"] - r + DOUT // 2].bitcast(bf16)
            degb_sb = crest_sb[0:1, o["degb"] - r:
                               o["degb"] - r + WPC * P // 2].bitcast(bf16)

            goff16 = 0
            for g0, Gg in _groups():
                xg = xgpool.tile([P, Gg * T * P], bf16, tag="xg")
                xg3 = xg[:].rearrange("p (c e) -> p c e", e=P)
                # four gathers per group on SWDGE queues 1,2,3,0: queue 1-3
                # instructions retire immediately (desc-gen runs async on
                # their own Q7 core pairs); only queue 0 blocks the engine.
                # The fixed period-4 queue cycle keeps Tile's 8 DMASW sem
                # lanes queue-consistent (lane j%8 always sees queue
                # cycle[j%4]).
                for src_d, cbase, c0, c1, q in _gsplits(Gg, T_LO, T_HI):
                    ni = (c1 - c0) * P
                    nc.gpsimd.dma_gather(
                        out_ap=xg3[:, cbase + c0:cbase + c1, :],
                        in_ap=(xlo_d if src_d == 0 else xhi_d)[:, :],
                        idxs_ap=idx16_sb[:, goff16:goff16 + ni // 16],
                        num_idxs=ni,
                        num_idxs_reg=ni,
                        elem_size=DIN,
                        single_packet=False,
                        queue_num=q,
                    )
                    goff16 += ni // 16
                for wl in range(Gg):
                    w = g0 + wl
                    # all T one-hots for this window in one wide DVE op:
                    # oh[p, t, j] = (iota[j] == col_local[p, t])
                    oh = ohpool.tile([P, T * P], bf16, tag="oh")
                    nc.vector.tensor_tensor(
                        out=oh[:].rearrange("p (t j) -> p t j", j=P),
                        in0=iotab_sb[:].rearrange("p (t j) -> p t j", j=P),
                        in1=colb_sb[:, w * T:(w + 1) * T].to_broadcast(
                            [P, T, P]),
                        op=mybir.AluOpType.is_equal,
                    )
                    agg_ps = pspool.tile([P, P], f32, tag="agg")
                    for t in range(T):
                        if t < T_LO:
                            c = wl * T_LO + t
                        else:
                            c = Gg * T_LO + wl * T_HI + (t - T_LO)
                        nc.tensor.matmul(
                            out=agg_ps[:],
                            lhsT=xg[:, c * P:(c + 1) * P],
                            rhs=oh[:, t * P:(t + 1) * P],
                            start=(t == 0),
                            stop=(t == T - 1),
                        )
                    aggT_sb = eppool.tile([P, P], bf16, tag="aggT")
                    nc.scalar.copy(out=aggT_sb[:], in_=agg_ps[:])
                    out_ps = popool.tile([P, DOUT], f32, tag="outp")
                    nc.tensor.matmul(out=out_ps[:], lhsT=aggT_sb[:],
                                     rhs=wvtb_sb[:], start=True, stop=False)
                    nc.tensor.matmul(out=out_ps[:],
                                     lhsT=degb_sb[0:1, w * P:(w + 1) * P],
                                     rhs=bvb_sb[0:1, :], start=False,
                                     stop=True)
                    out_sb = eppool.tile([P, DOUT], f32, tag="outs")
                    nc.scalar.mul(out=out_sb[:], in_=out_ps[:],
                                  mul=rec_sb[:, w:w + 1])
                    nc.sync.dma_start(out=out_d[w * P:(w + 1) * P, :],
                                      in_=out_sb[:])
    nc.compile()
    # Rewrite each gather's SWDGE queue as a pure function of its ASSIGNED
    # DMASW sem lane, so every lane is incremented by exactly one queue
    # (the ucode tracks sem ownership per queue).  Queue 1-3 instructions
    # retire immediately on the Pool engine (desc-gen runs on their own Q7
    # core pairs); queue 0 blocks, so it gets 2 of the 8 lanes.
    lane_q = (1, 2, 3, 0)
    for bb in nc.m.functions[0].blocks:
        for inst in bb.instructions:
            if 'DMAGatherAnt' not in type(inst).__name__:
                continue
            lane = None
            si = inst.sync_info
            if si is not None:
                for u in si.on_update:
                    n = u.ant_name
                    if n and n.startswith('DMASW'):
                        lane = int(n[5:].split('_')[0])
            assert lane is not None, "gather without DMASW sem"
            inst.queue_num = lane_q[lane % 4]
    return nc


def _prep(row, col):
    """Host-side packing. Returns (T, T_LO, T_HI, per-core arrays)."""
    row = row.astype(np.int64)
    col = col.astype(np.int64)
    E = len(row)
    ishi = (row >= XLO).astype(np.int64)
    key = ((col >> 7) << 1) | ishi
    order = np.argsort(key, kind="stable")
    srow = row[order]
    scol = col[order]
    skey = key[order]

    deg = np.bincount(col, minlength=NPAD).astype(np.float32)
    recip = np.where(deg > 0, 1.0 / np.maximum(deg, 1.0), 0.0).astype(np.float32)

    cnt = np.bincount(key, minlength=2 * NWIN)
    lo_cnt, hi_cnt = cnt[0::2], cnt[1::2]
    T_LO = int(np.ceil(lo_cnt.max() / P))
    T_HI = int(np.ceil(hi_cnt.max() / P))
    T = T_LO + T_HI

    gstart = np.zeros(2 * NWIN + 1, np.int64)
    np.cumsum(cnt, out=gstart[1:])
    epos = np.arange(E, dtype=np.int64) - gstart[skey]
    p = epos % P
    tw = epos // P
    whalf = skey & 1
    win = skey >> 1
    tchunk = np.where(whalf == 1, tw + T_LO, tw)

    col_arr = np.full((NWIN, P, T), -1.0, np.float32)
    col_arr[win, p, tchunk] = (scol & (P - 1)).astype(np.float32)

    idx_lo = np.zeros((NWIN, T_LO * P), np.int16)
    idx_hi = np.zeros((NWIN, T_HI * P), np.int16)
    lo_m = whalf == 0
    hi_m = whalf == 1
    idx_lo[win[lo_m], epos[lo_m]] = srow[lo_m].astype(np.int16)
    idx_hi[win[hi_m], epos[hi_m]] = (srow[hi_m] - XLO).astype(np.int16)

    per_core = []
    for c in range(NCORES):
        wsl = slice(c * WPC, (c + 1) * WPC)
        # wrapped idx16 layout: per gather, index i at [i%16, i//16],
        # replicated across the 8 groups of 16 partitions; four gather
        # blocks per group matching _gsplits order
        cols16 = []
        for g0, Gg in _groups():
            wabs = c * WPC + g0
            halves = (idx_lo[wabs:wabs + Gg].reshape(-1),
                      idx_hi[wabs:wabs + Gg].reshape(-1))
            for src_d, cbase, c0, c1, q in _gsplits(Gg, T_LO, T_HI):
                flat = halves[src_d][c0 * P:c1 * P]
                wrapped = flat.reshape(-1, 16).T             # [16, ni/16]
                cols16.append(np.tile(wrapped, (8, 1)))      # [128, ni/16]
        idx16_map = np.concatenate(cols16, axis=1)           # [128, WPC*T*8]
        col_map = np.ascontiguousarray(
            col_arr[wsl].transpose(1, 0, 2).reshape(P, WPC * T))
        rec_map = np.ascontiguousarray(
            recip[c * WPC * P:(c + 1) * WPC * P].reshape(WPC, P).T)
        deg_map = np.ascontiguousarray(
            deg[c * WPC * P:(c + 1) * WPC * P].reshape(1, WPC * P))
        per_core.append((idx16_map, col_map, rec_map, deg_map))
    return T, T_LO, T_HI, per_core


def _put_bf16(arr, col_off, data_bf16):
    """Pack a bf16 [rows, n] block into f32 columns of arr at col_off."""
    rows, n = data_bf16.shape
    assert n % 2 == 0
    tmp = np.zeros((rows, n // 2), np.float32)
    tmp.view(np.uint16).reshape(rows, n)[:] = data_bf16.view(np.uint16)
    arr[:rows, col_off:col_off + n // 2] = tmp


def _pack_const(T, idx16_map, col_map, rec_map, deg_map, wvtb, bvb):
    """Returns (cidx, crest) arrays for the two constant tensors."""
    from ml_dtypes import bfloat16
    o = _offsets(T)
    r = WPC * T * 4
    assert idx16_map.shape == (P, WPC * T * 8)
    cidx = np.ascontiguousarray(idx16_map.view(np.float32))
    arr = np.zeros((P, o["CW"] - r), np.float32)
    _put_bf16(arr, o["colb"] - r, col_map.astype(bfloat16))
    arr[:, o["rec"] - r:o["rec"] - r + WPC] = rec_map
    _put_bf16(arr, o["wvtb"] - r, wvtb)
    iotab = np.broadcast_to(
        np.tile(np.arange(P, dtype=np.float32), T)[None, :],
        (P, T * P)).astype(bfloat16)
    _put_bf16(arr, o["iotab"] - r, np.ascontiguousarray(iotab))
    _put_bf16(arr, o["bvb"] - r, bvb)
    _put_bf16(arr, o["degb"] - r, deg_map.astype(bfloat16))
    return cidx, arr


def kernel(**inputs):
    global _last_exec_ns
    _ensure_ntff_hook()
    from concourse.bass_utils import run_bass_kernel_spmd
    from ml_dtypes import bfloat16

    x = np.ascontiguousarray(np.asarray(inputs["x"], dtype=np.float32))
    ei = np.asarray(inputs["edge_index"])
    row = np.asarray(ei[0]).astype(np.int64)
    col = np.asarray(ei[1]).astype(np.int64)
    Wv = np.asarray(inputs["Wv"], dtype=np.float32)
    bv = np.asarray(inputs["bv"], dtype=np.float32)

    xb = x.astype(bfloat16)
    wvtb = np.ascontiguousarray(Wv.T).astype(bfloat16)     # [DIN, DOUT]
    bvb = bv.reshape(1, DOUT).astype(bfloat16)

    T, T_LO, T_HI, per_core = _prep(row, col)

    key = (T, T_LO, T_HI)
    if key not in _cache:
        _cache[key] = _build(T, T_LO, T_HI)
    nc = _cache[key]

    xlo = np.ascontiguousarray(xb[:XLO])
    xhi = np.ascontiguousarray(xb[XLO:])
    in_maps = []
    for c in range(NCORES):
        cidx, crest = _pack_const(T, *per_core[c], wvtb, bvb)
        in_maps.append({"xlo": xlo, "xhi": xhi, "cidx": cidx,
                        "crest": crest})

    trace = bool(os.environ.get("GAT_TRACE"))
    res = run_bass_kernel_spmd(nc, in_maps, list(range(NCORES)), trace=trace)
    _last_exec_ns = res.exec_time_ns
    globals()["_last_res"] = res

    out = np.concatenate([res.results[c]["out"] for c in range(NCORES)], axis=0)
    return np.ascontiguousarray(out[:N])



# revision 4
# speedup vs baseline: 1.1101x; 1.1101x over previous
"""GAT layer kernel for Trainium2 (8 NeuronCores, SPMD) — bf16 pipeline, V2.

Math note: the per-destination softmax weights are only used through their
mean over each destination's incoming edges, and a softmax sums to 1, so
attn_w[i] = 1/deg[i] (0 if deg==0) exactly.  The output reduces to:

    out[i] = (agg[i] @ Wv.T + deg[i]*bv) * recip[i],  agg[i] = sum x[row[e]]

Device strategy (dst-node sharded, 49 windows of 128 dst nodes per core):
  - host sorts edges by (group, src half, window) and packs each group's
    edge list CONTIGUOUSLY across window boundaries: 128-slot chunks are
    shared between adjacent windows (the one-hot masks foreign slots
    with col=-1; boundary chunks are accumulated by both windows).  Per
    (group, half) the chunk count is the max across the 8 cores (SPMD
    needs one program), idx-0 padded — ~4% slack vs per-core exact.
    SWDGE descriptor generation (~8ns/row/queue on the 4 Q7 queue
    pairs) is the critical path, so descriptor count ~= packed count.
  - per group of G windows: FOUR dma_gather calls (int16 indices, x
    split into two <32768-row halves) on SWDGE queues 1,2,3,0.  Index
    tables are DMA'd just-in-time per group on the Sync queue, which
    carries nothing else, so prefetch never blocks — this removes the
    21us startup bubble the full-table preload had.
  - per window one wide DVE op builds the one-hots; TensorE accumulates
    aggT[din, dst] += Xg_c^T @ oh_t into PSUM with bf16 matmuls.
  - epilogues (out[dst,:] = (aggT^T @ WvT + deg^T x bv) * recip) are
    DEFERRED one group: their matmuls are issued between groups when
    their inputs are long ready, so TensorE never stalls on the Scalar
    PSUM->SBUF round trip (was ~0.8us/window in the drain).  Scalar
    carries the aggT copies, the recip scale, and the output DMAs.
"""

import os
import numpy as np

P = 128
NCORES = 8
N = 50000
XLO = 25088                   # rows in the low half of x (< 32768 for int16)
XHI = N - XLO
DIN = 128
DOUT = 128
WPC = 49                      # windows per core
NWIN = NCORES * WPC           # 392
NPAD = NWIN * P               # 50176

_last_exec_ns = None
_cache = {}


def _group_sizes():
    # tapered tail: small final groups drain the pipeline quickly
    return [5] * 9 + [2, 1, 1]


def _ensure_ntff_hook():
    import sys
    import types
    if "antenv.axon_hooks" in sys.modules:
        return
    try:
        import antenv
        mod = types.ModuleType("antenv.axon_hooks")
        _h = [None]
        mod.set_axon_ntff_profile_hook = lambda hook: _h.__setitem__(0, hook)
        mod.get_axon_ntff_profile_hook = lambda: _h[0]
        sys.modules["antenv.axon_hooks"] = mod
        antenv.axon_hooks = mod
        from trn_agent_boot.trn_boot import _ntff_profile_via_ctypes
        hook = _ntff_profile_via_ctypes("/opt/axon/libaxon_pjrt.so")
        if hook is not None:
            mod.set_axon_ntff_profile_hook(hook)
    except Exception:
        pass


class Layout:
    """Compile-time (data-dependent, core-common) packing.

    groups: list of dicts with
      nchunks: total xg chunks C_g
      gathers: list of (src_half, cbase_chunks, nchunks) in issue order
      windows: list of (chunklist, colb_off); chunklist = absolute xg
               chunk ids the window accumulates (union across cores)
    tmax: max T_w;  ncid: f32 cols of cidx;  ncolb: colb columns
    idx_f32_off: per-group first f32 column in cidx (+ final sentinel)
    """

    def __init__(self):
        self.groups = []
        self.tmax = 0
        self.ncid = 0
        self.ncolb = 0
        self.idx_f32_off = []

    def key(self):
        parts = [self.tmax, self.ncid, self.ncolb, tuple(self.idx_f32_off)]
        for g in self.groups:
            parts.append((g["nchunks"], tuple(g["gathers"]),
                          tuple((tuple(cl), off) for cl, off in g["windows"])))
        return hash(str(parts))


def _prep(row, col):
    """Host-side packing. Returns (lay, per_core arrays)."""
    row = row.astype(np.int64)
    col = col.astype(np.int64)
    ishi = (row >= XLO).astype(np.int64)

    deg = np.bincount(col, minlength=NPAD).astype(np.float32)
    recip = np.where(deg > 0, 1.0 / np.maximum(deg, 1.0), 0.0).astype(np.float32)

    sizes = _group_sizes()
    NG = len(sizes)
    g0s = np.concatenate([[0], np.cumsum(sizes)[:-1]])

    win = col >> 7
    core = win // WPC
    wloc = win - core * WPC
    dloc = (col & (P - 1)).astype(np.int64)
    wl2g = np.zeros(WPC, np.int64)
    for gi in range(NG):
        wl2g[g0s[gi]:g0s[gi] + sizes[gi]] = gi

    order = np.lexsort((wloc, ishi, wl2g[wloc], core))
    srow, score, shalf, swloc, sd = (row[order], core[order], ishi[order],
                                     wloc[order], dloc[order])
    sg = wl2g[swloc]

    # segment pointers per (core, group, half)
    seg_key = (score * NG + sg) * 2 + shalf
    seg_cnt = np.bincount(seg_key, minlength=NCORES * NG * 2)
    seg_start = np.zeros(NCORES * NG * 2 + 1, np.int64)
    np.cumsum(seg_cnt, out=seg_start[1:])

    lay = Layout()
    # chunk counts per (group, half) = max over cores
    Ch = np.zeros((NG, 2), np.int64)
    for gi in range(NG):
        for h in (0, 1):
            n_max = max(seg_cnt[(c * NG + gi) * 2 + h] for c in range(NCORES))
            Ch[gi, h] = max(1, -(-n_max // P))

    # build groups metadata + per-core data
    idx16_cols = []          # list of per-core [128, ni/16] blocks, per gather
    colb_cols = []           # list of per-core [128] col arrays, per column
    colb_off = 0
    for gi in range(NG):
        C_lo, C_hi = int(Ch[gi, 0]), int(Ch[gi, 1])
        cbase_h = (0, C_lo)
        gathers = []
        for h in (0, 1):
            Chh = (C_lo, C_hi)[h]
            assert Chh >= 2, f"half too small: g{gi} h{h} Chh={Chh}"
            ca = (Chh + 1) // 2
            for sp in ((0, ca), (ca, Chh)):
                gathers.append((h, cbase_h[h] + sp[0], sp[1] - sp[0]))
        # order is (lo A, lo B, hi A, hi B) -> queues 1,2,3,0
        # per-core idx data per gather
        for (h, cb, nchk) in gathers:
            c0 = cb - cbase_h[h]
            blocks = []
            for c in range(NCORES):
                s = seg_start[(c * NG + gi) * 2 + h]
                n = seg_cnt[(c * NG + gi) * 2 + h]
                v = np.zeros(nchk * P, np.int16)
                lo_s, hi_s = c0 * P, c0 * P + nchk * P
                take0, take1 = min(lo_s, n), min(hi_s, n)
                nn = take1 - take0
                if nn > 0:
                    v[:nn] = (srow[s + take0:s + take1]
                              - (XLO if h else 0)).astype(np.int16)
                wrapped = v.reshape(-1, 16).T            # [16, ni/16]
                blocks.append(np.tile(wrapped, (8, 1)))  # [128, ni/16]
            idx16_cols.append(blocks)

        # per-window union spans + col data
        wins = []
        for wl in range(sizes[gi]):
            spans = {0: [None, None], 1: [None, None]}
            percore_rng = np.zeros((NCORES, 2, 2), np.int64)  # [c,h,(s0,s1)]
            for h in (0, 1):
                lo_c, hi_c = None, None
                for c in range(NCORES):
                    s = seg_start[(c * NG + gi) * 2 + h]
                    n = seg_cnt[(c * NG + gi) * 2 + h]
                    wseg = swloc[s:s + n] - g0s[gi]
                    idxs = np.flatnonzero(wseg == wl)
                    if len(idxs) == 0:
                        percore_rng[c, h] = (0, 0)
                        continue
                    s0, s1 = int(idxs[0]), int(idxs[-1]) + 1
                    percore_rng[c, h] = (s0, s1)
                    a, b = s0 // P, -(-s1 // P)
                    lo_c = a if lo_c is None else min(lo_c, a)
                    hi_c = b if hi_c is None else max(hi_c, b)
                spans[h] = [lo_c, hi_c]
            chunklist = []
            ncols = 0
            for h in (0, 1):
                lo_c, hi_c = spans[h]
                if lo_c is None:
                    continue
                for cc in range(lo_c, hi_c):
                    chunklist.append(cbase_h[h] + cc)
                    colv_pc = []
                    for c in range(NCORES):
                        colv = np.full(P, -1.0, np.float32)
                        s = seg_start[(c * NG + gi) * 2 + h]
                        s0, s1 = percore_rng[c, h]
                        if s1 > s0:
                            a = max(s0, cc * P)
                            b = min(s1, (cc + 1) * P)
                            if b > a:
                                colv[a - cc * P:b - cc * P] = \
                                    sd[s + a:s + b].astype(np.float32)
                        colv_pc.append(colv)
                    colb_cols.append(colv_pc)
                    ncols += 1
            assert ncols >= 1, f"empty window g{gi} w{wl}"
            wins.append((chunklist, colb_off))
            colb_off += ncols
            lay.tmax = max(lay.tmax, ncols)
        lay.groups.append({"nchunks": C_lo + C_hi,
                           "gathers": gathers, "windows": wins})

    # cidx assembly: per group blocks, f32 offsets
    off = 0
    per_core_idx = [[] for _ in range(NCORES)]
    bi = 0
    for gi in range(NG):
        lay.idx_f32_off.append(off)
        for _ in lay.groups[gi]["gathers"]:
            blocks = idx16_cols[bi]
            bi += 1
            for c in range(NCORES):
                per_core_idx[c].append(blocks[c])
            off += blocks[0].shape[1] // 2
    lay.idx_f32_off.append(off)
    lay.ncid = off
    lay.ncolb = colb_off

    per_core = []
    for c in range(NCORES):
        idx16 = np.concatenate(per_core_idx[c], axis=1)    # [128, 2*ncid]
        col_map = np.stack([pc[c] for pc in colb_cols], axis=1)  # [128,ncolb]
        rec_map = np.ascontiguousarray(
            recip[c * WPC * P:(c + 1) * WPC * P].reshape(WPC, P).T)
        deg_map = np.ascontiguousarray(
            deg[c * WPC * P:(c + 1) * WPC * P].reshape(1, WPC * P))
        per_core.append((idx16, col_map, rec_map, deg_map))
    return lay, per_core


def _offsets(lay):
    """Column offsets of the packed [P, CW] f32 crest tensor."""
    o = {}
    o["colb"] = 0                                   # bf16, ncolb cols
    o["rec"] = o["colb"] + (lay.ncolb + 1) // 2
    o["wvtb"] = o["rec"] + WPC
    o["iotab"] = o["wvtb"] + DOUT // 2
    o["bvb"] = o["iotab"] + lay.tmax * P // 2
    o["degb"] = o["bvb"] + DOUT // 2
    o["CW"] = o["degb"] + WPC * P // 2
    return o


def _build(lay):
    import concourse.bacc as bacc
    import concourse.mybir as mybir
    from concourse.tile import TileContext

    f32 = mybir.dt.float32
    bf16 = mybir.dt.bfloat16
    i16 = mybir.dt.int16

    o = _offsets(lay)
    CW = o["CW"]

    nc = bacc.Bacc(None, target_bir_lowering=False, num_swdge_queues=4)
    xlo_d = nc.dram_tensor("xlo", [XLO, DIN], bf16, kind="ExternalInput")
    xhi_d = nc.dram_tensor("xhi", [XHI, DIN], bf16, kind="ExternalInput")
    cidx_d = nc.dram_tensor("cidx", [P, lay.ncid], f32, kind="ExternalInput")
    crest_d = nc.dram_tensor("crest", [P, CW], f32, kind="ExternalInput")
    out_d = nc.dram_tensor("out", [WPC * P, DOUT], f32, kind="ExternalOutput")

    sizes = _group_sizes()
    g0s = np.concatenate([[0], np.cumsum(sizes)[:-1]])
    QCYC = (1, 2, 3, 0)

    with TileContext(nc) as tc:
        with (
            tc.tile_pool(name="const", bufs=1) as cpool,
            tc.tile_pool(name="idx", bufs=3) as ipool,
            tc.tile_pool(name="xg", bufs=6) as xgpool,
            tc.tile_pool(name="oh", bufs=6) as ohpool,
            tc.tile_pool(name="at", bufs=12) as atpool,
            tc.tile_pool(name="os", bufs=4) as ospool,
            tc.tile_pool(name="ps", bufs=2, space="PSUM") as pspool,
            tc.tile_pool(name="po", bufs=4, space="PSUM") as popool,
        ):
            crest_sb = cpool.tile([P, CW], f32, tag="crest")
            nc.sync.dma_start(out=crest_sb[:], in_=crest_d[:, :])

            colb_sb = crest_sb[:, o["colb"]:o["rec"]].bitcast(bf16)
            rec_sb = crest_sb[:, o["rec"]:o["rec"] + WPC]
            wvtb_sb = crest_sb[:, o["wvtb"]:o["wvtb"] + DOUT // 2].bitcast(bf16)
            iotab_sb = crest_sb[:, o["iotab"]:
                                o["iotab"] + lay.tmax * P // 2].bitcast(bf16)
            bvb_sb = crest_sb[0:1, o["bvb"]:o["bvb"] + DOUT // 2].bitcast(bf16)
            degb_sb = crest_sb[0:1, o["degb"]:
                               o["degb"] + WPC * P // 2].bitcast(bf16)

            def epilogue(wl_abs, aggT_sb):
                out_ps = popool.tile([P, DOUT], f32, tag="outp")
                nc.tensor.matmul(out=out_ps[:], lhsT=aggT_sb[:],
                                 rhs=wvtb_sb[:], start=True, stop=False)
                nc.tensor.matmul(out=out_ps[:],
                                 lhsT=degb_sb[0:1, wl_abs * P:(wl_abs + 1) * P],
                                 rhs=bvb_sb[0:1, :], start=False, stop=True)
                out_sb = ospool.tile([P, DOUT], f32, tag="outs")
                nc.scalar.mul(out=out_sb[:], in_=out_ps[:],
                              mul=rec_sb[:, wl_abs:wl_abs + 1])
                nc.scalar.dma_start(
                    out=out_d[wl_abs * P:(wl_abs + 1) * P, :], in_=out_sb[:])

            pending = []          # deferred (wl_abs, aggT_sb) epilogues
            for gi, g in enumerate(lay.groups):
                C = g["nchunks"]
                f0, f1 = lay.idx_f32_off[gi], lay.idx_f32_off[gi + 1]
                idx_sb = ipool.tile([P, f1 - f0], f32, tag="idx")
                nc.sync.dma_start(out=idx_sb[:], in_=cidx_d[:, f0:f1])
                idx16_sb = idx_sb[:].bitcast(i16)

                xg = xgpool.tile([P, C * P], bf16, tag="xg")
                xg3 = xg[:].rearrange("p (c e) -> p c e", e=P)
                goff = 0
                for k, (h, cb, nchk) in enumerate(g["gathers"]):
                    ni = nchk * P
                    nc.gpsimd.dma_gather(
                        out_ap=xg3[:, cb:cb + nchk, :],
                        in_ap=(xlo_d if h == 0 else xhi_d)[:, :],
                        idxs_ap=idx16_sb[:, goff:goff + ni // 16],
                        num_idxs=ni,
                        num_idxs_reg=ni,
                        elem_size=DIN,
                        single_packet=False,
                        queue_num=QCYC[k % 4],
                    )
                    goff += ni // 16

                # flush previous group's epilogues (inputs long ready)
                for wl_abs, at in pending:
                    epilogue(wl_abs, at)
                pending = []

                for wl, (chunklist, coff) in enumerate(g["windows"]):
                    wl_abs = int(g0s[gi]) + wl
                    T_w = len(chunklist)
                    oh = ohpool.tile([P, T_w * P], bf16, tag="oh")
                    nc.vector.tensor_tensor(
                        out=oh[:].rearrange("p (t j) -> p t j", j=P),
                        in0=iotab_sb[:, :T_w * P].rearrange(
                            "p (t j) -> p t j", j=P),
                        in1=colb_sb[:, coff:coff + T_w].to_broadcast(
                            [P, T_w, P]),
                        op=mybir.AluOpType.is_equal,
                    )
                    agg_ps = pspool.tile([P, P], f32, tag="agg")
                    for t, xc in enumerate(chunklist):
                        nc.tensor.matmul(
                            out=agg_ps[:],
                            lhsT=xg[:, xc * P:(xc + 1) * P],
                            rhs=oh[:, t * P:(t + 1) * P],
                            start=(t == 0),
                            stop=(t == T_w - 1),
                        )
                    aggT_sb = atpool.tile([P, P], bf16, tag="aggT")
                    nc.scalar.copy(out=aggT_sb[:], in_=agg_ps[:])
                    pending.append((wl_abs, aggT_sb))
            for wl_abs, at in pending:
                epilogue(wl_abs, at)
    nc.compile()
    # Rewrite each gather's SWDGE queue as a pure function of its ASSIGNED
    # DMASW sem lane, so every lane is incremented by exactly one queue
    # (the ucode tracks sem ownership per queue).
    lane_q = (1, 2, 3, 0)
    for bb in nc.m.functions[0].blocks:
        for inst in bb.instructions:
            if 'DMAGatherAnt' not in type(inst).__name__:
                continue
            lane = None
            si = inst.sync_info
            if si is not None:
                for u in si.on_update:
                    n = u.ant_name
                    if n and n.startswith('DMASW'):
                        lane = int(n[5:].split('_')[0])
            assert lane is not None, "gather without DMASW sem"
            inst.queue_num = lane_q[lane % 4]
    return nc


def _put_bf16(arr, col_off, data_bf16):
    """Pack a bf16 [rows, n] block into f32 columns of arr at col_off."""
    rows, n = data_bf16.shape
    if n % 2:
        data_bf16 = np.concatenate(
            [data_bf16, np.zeros((rows, 1), data_bf16.dtype)], axis=1)
        n += 1
    tmp = np.zeros((rows, n // 2), np.float32)
    tmp.view(np.uint16).reshape(rows, n)[:] = data_bf16.view(np.uint16)
    arr[:rows, col_off:col_off + n // 2] = tmp


def _pack_const(lay, idx16, col_map, rec_map, deg_map, wvtb, bvb):
    """Returns (cidx, crest) arrays for the two constant tensors."""
    from ml_dtypes import bfloat16
    o = _offsets(lay)
    assert idx16.shape == (P, lay.ncid * 2), idx16.shape
    cidx = np.ascontiguousarray(idx16).view(np.float32)
    arr = np.zeros((P, o["CW"]), np.float32)
    _put_bf16(arr, o["colb"], col_map.astype(bfloat16))
    arr[:, o["rec"]:o["rec"] + WPC] = rec_map
    _put_bf16(arr, o["wvtb"], wvtb)
    iotab = np.broadcast_to(
        np.tile(np.arange(P, dtype=np.float32), lay.tmax)[None, :],
        (P, lay.tmax * P)).astype(bfloat16)
    _put_bf16(arr, o["iotab"], np.ascontiguousarray(iotab))
    _put_bf16(arr, o["bvb"], bvb)
    _put_bf16(arr, o["degb"], deg_map.astype(bfloat16))
    return cidx, arr


def kernel(**inputs):
    global _last_exec_ns
    _ensure_ntff_hook()
    from concourse.bass_utils import run_bass_kernel_spmd
    from ml_dtypes import bfloat16

    x = np.ascontiguousarray(np.asarray(inputs["x"], dtype=np.float32))
    ei = np.asarray(inputs["edge_index"])
    row = np.asarray(ei[0]).astype(np.int64)
    col = np.asarray(ei[1]).astype(np.int64)
    Wv = np.asarray(inputs["Wv"], dtype=np.float32)
    bv = np.asarray(inputs["bv"], dtype=np.float32)

    xb = x.astype(bfloat16)
    wvtb = np.ascontiguousarray(Wv.T).astype(bfloat16)     # [DIN, DOUT]
    bvb = bv.reshape(1, DOUT).astype(bfloat16)

    lay, per_core = _prep(row, col)

    key = lay.key()
    if key not in _cache:
        _cache[key] = _build(lay)
    nc = _cache[key]

    xlo = np.ascontiguousarray(xb[:XLO])
    xhi = np.ascontiguousarray(xb[XLO:])
    in_maps = []
    for c in range(NCORES):
        cidx, crest = _pack_const(lay, *per_core[c], wvtb, bvb)
        in_maps.append({"xlo": xlo, "xhi": xhi, "cidx": cidx,
                        "crest": crest})

    trace = bool(os.environ.get("GAT_TRACE"))
    res = run_bass_kernel_spmd(nc, in_maps, list(range(NCORES)), trace=trace)
    _last_exec_ns = res.exec_time_ns
    globals()["_last_res"] = res

    out = np.concatenate([res.results[c]["out"] for c in range(NCORES)], axis=0)
    return np.ascontiguousarray(out[:N])
